# revision 23
# baseline (speedup 1.0000x reference)
"""DirichletLoss kernel for 8 trn2 NeuronCores.

Math: per graph b, per channel d:
    de[d] = f_d^T L f_d  with f = row-normalized h.

v5 (fp8 one-pass, host-prepacked operands): L is quantized RAW to fp8
e4m3 on the host (4x less HBM traffic; raw N(0,1) entries sit in
e4m3's sweet spot -- folding the norms into L instead lands in the
denormal range and costs 3x accuracy). Normalization is folded into the
stationary side on the host: f16 = fp16(8 * h/||h||row), split into two
fp8 planes (q8a = e4m3(f16), q8b = e4m3(f16 - q8a)) to cancel the
stationary quantization error; the x8 pre-scale keeps both planes out
of e4m3 denormals. Host ships the planes pre-packed [p, k, two, d] and
f^T pre-transposed [64, 2048] f16, so the device does NO casts and NO
PE transposes -- its PE stream is exactly 128 DoubleRow matmuls: both
planes contract against each L row in a single pass via a stride-0
moving AP (out[d,c] = sum_p (q8a+q8b)[p,d] L[p,c]). That is the pure
column floor, 1 column/cycle @2.4GHz = 13.9us/side (measured: weight
switches and moving-tile switches are free; DoubleRow adds rows per
instruction, not column rate; last-slab cadence measured 216ns).

Epilogue: fused multiply+reduce on DVE per PSUM bank (only DVE reads
PSUM), host divides by 8^2 and finishes the masked mean. End-to-end
error is the deterministic L-quant term (~6e-3 rel on the fixed seed
vs the 2e-2 gate, host-emulated exactly). Sharding: graph b -> core b.
"""

import numpy as np
import ml_dtypes

import concourse.bacc as bacc
import concourse.bass as bass
import concourse.mybir as mybir
import concourse.tile as tile
from concourse.bass_utils import run_bass_kernel_spmd

B = 8
N = 2048
D = 64
P = 128
NCHUNK = N // P   # 16 contraction chunks of 128 rows
MM_N = 512        # PSUM bank: 512 f32 out columns per matmul
NI = N // MM_N    # 4 output column blocks
F32 = mybir.dt.float32
F16 = mybir.dt.float16
FP8 = mybir.dt.float8e4
E4NP = ml_dtypes.float8_e4m3

# --- tunables -------------------------------------------------------------
SLAB_BUFS = 10  # one buffer per slab: no recycling waits on the sync queue
# Ramp-up slab sizes: small first slabs so the first matmuls start as soon
# as possible (the PE stream is gap-free and ends at start + 30.6us, so
# every us of earlier start is a us off the total).
SLABS = [128, 256, 512, 512, 640]
F_SCALE = 8.0     # keeps f16 and both fp8 planes out of e4m3 denormals
# --------------------------------------------------------------------------


def _emit_body(nc, tc, pools, aps):
    (qpool, ftpool, slabpool, psumpool, smallpool, outpool) = pools
    Ls, q8s, fTs, Lt, q8t, fTt, out = aps

    # accum targets: column side*4 + bank; host sums the 4 pieces per side
    out_sb = outpool.tile([D, 8], F32, tag="out_sb")

    side_small = ((q8s, fTs), (q8t, fTt))

    for side, L_ap in enumerate((Ls, Lt)):
        # small operands issue on the (idle) ACT queue, concurrent with the
        # sync queue's slab issues
        q_ap, fT_ap = side_small[side]
        f8 = qpool.tile([P, NCHUNK, 2, D], FP8, tag="f8")
        nc.scalar.dma_start(
            out=f8[:].rearrange("p k two d -> p (k two d)"), in_=q_ap[:, :]
        )
        fT_sb = ftpool.tile([D, N], F16, tag="fT_sb")
        nc.scalar.dma_start(out=fT_sb[:], in_=fT_ap[:, :])
        P_ps = psumpool.tile([D, N], F32, tag="ps", name="P_ps")
        row0 = 0
        for si, rows in enumerate(SLABS):
            n_blk = rows // P
            slab = slabpool.tile([P, n_blk * N], FP8, tag="slab")
            # L is host-packed [p, k, c] (k = row-block), so a slab is one
            # contiguous 8KB-per-partition descriptor run. All slabs issue
            # on the sync(SP) queue: per-side or alternating splits across
            # queues measured consistently worse.
            k0 = row0 // P
            nc.sync.dma_start(
                out=slab[:],
                in_=L_ap[:, k0 * N : (k0 + n_blk) * N],
            )
            for n in range(n_blk):
                k = row0 // P + n  # global chunk index
                for i in range(NI):
                    # stride-0 "two" dim: both planes contract the same rows
                    mv = slab[:, n * N + i * MM_N : n * N + (i + 1) * MM_N]
                    mv_b = bass.AP(
                        mv.tensor, mv.offset,
                        [list(mv.ap[0]), [0, 2], [1, MM_N]],
                    )
                    nc.tensor.matmul(
                        P_ps[:, i * MM_N : (i + 1) * MM_N],
                        f8[:, k, :, :],
                        mv_b,
                        start=(k == 0),
                        stop=(k == NCHUNK - 1),
                        perf_mode=mybir.MatmulPerfMode.DoubleRow,
                    )
            row0 += rows

        # de pieces: one fused DVE op per PSUM bank via
        # scalar_tensor_tensor accum_out, straight into out_sb (gpsimd
        # cannot access PSUM; only DVE reads it for tensor ops). NOTE: the
        # dedicated tensor_tensor_reduce op crashes the exec unit on HW;
        # the TensorScalarPtr accum path is the safe fused form.
        for i in range(NI):
            tmp = smallpool.tile([D, MM_N], F32, tag=f"ttr_tmp{i % 2}",
                                 name=f"tmp{i}")
            nc.vector.scalar_tensor_tensor(
                out=tmp[:],
                in0=P_ps[:, i * MM_N : (i + 1) * MM_N],
                scalar=1.0,
                in1=fT_sb[:, i * MM_N : (i + 1) * MM_N],
                op0=mybir.AluOpType.mult,
                op1=mybir.AluOpType.mult,
                accum_out=out_sb[:, 4 * side + i : 4 * side + i + 1],
            )

    nc.sync.dma_start(out=out[:], in_=out_sb[:])


def build_program():
    nc = bacc.Bacc(trn_type="TRN2")

    Ls = nc.declare_dram_parameter("Ls", [P, NCHUNK * N], FP8, isOutput=False)
    q8s = nc.declare_dram_parameter("q8s", [P, NCHUNK * 2 * D], FP8, isOutput=False)
    fTs = nc.declare_dram_parameter("fTs", [D, N], F16, isOutput=False)
    Lt = nc.declare_dram_parameter("Lt", [P, NCHUNK * N], FP8, isOutput=False)
    q8t = nc.declare_dram_parameter("q8t", [P, NCHUNK * 2 * D], FP8, isOutput=False)
    fTt = nc.declare_dram_parameter("fTt", [D, N], F16, isOutput=False)
    out = nc.declare_dram_parameter("out", [D, 8], F32, isOutput=True)
    aps = (Ls, q8s, fTs, Lt, q8t, fTt, out)

    with tile.TileContext(nc) as tc:
        with (
            tc.tile_pool(name="qp", bufs=2) as qpool,
            tc.tile_pool(name="ftp", bufs=2) as ftpool,
            tc.tile_pool(name="slab", bufs=SLAB_BUFS) as slabpool,
            tc.tile_pool(name="psum", bufs=2, space="PSUM") as psumpool,
            tc.tile_pool(name="small", bufs=2) as smallpool,
            tc.tile_pool(name="outp", bufs=1) as outpool,
        ):
            pools = (qpool, ftpool, slabpool, psumpool, smallpool, outpool)
            _emit_body(nc, tc, pools, aps)

    nc.compile()
    return nc


_CACHED_NC = None


def _get_nc():
    global _CACHED_NC
    if _CACHED_NC is None:
        _CACHED_NC = build_program()
    return _CACHED_NC


def _prep_side(lap, h):
    """Per-graph host prep: Lq e4m3 packed [p, k, c], fp8 plane pair, fT f16."""
    Lq = np.asarray(lap, dtype=np.float32).reshape(B, N, N).astype(E4NP)
    # pack to the SBUF image: Lpk[b, p, k, c] = L[b, k*128+p, c]
    Lq = np.ascontiguousarray(
        Lq.reshape(B, NCHUNK, P, N).transpose(0, 2, 1, 3)
    ).reshape(B, P, NCHUNK * N)
    hh = np.asarray(h, dtype=np.float64).reshape(B, N, D)
    n = np.sqrt((hh * hh).sum(axis=2, keepdims=True))
    f16 = (hh / np.maximum(n, 1e-12) * F_SCALE).astype(np.float16)
    f32 = f16.astype(np.float32)
    q8a = f32.astype(E4NP)
    q8b = (f32 - q8a.astype(np.float32)).astype(E4NP)
    # pack [B, p, k, two, d]: row j = k*128 + p
    qab = np.stack([q8a, q8b], axis=2)            # [B, N, 2, D]
    qab = qab.reshape(B, NCHUNK, P, 2, D).transpose(0, 2, 1, 3, 4)
    q8 = np.ascontiguousarray(qab).reshape(B, P, NCHUNK * 2 * D)
    fT = np.ascontiguousarray(f16.transpose(0, 2, 1))  # [B, D, N]
    return Lq, q8, fT


def _shard_inputs(inputs):
    Lqs, q8s, fTs = _prep_side(inputs["laplacian_s"], inputs["h_s"])
    Lqt, q8t, fTt = _prep_side(inputs["laplacian_t"], inputs["h_t"])
    return [
        {
            "Ls": Lqs[b], "q8s": q8s[b], "fTs": fTs[b],
            "Lt": Lqt[b], "q8t": q8t[b], "fTt": fTt[b],
        }
        for b in range(B)
    ]


def _finish(core_outs, inputs):
    has_s = np.asarray(inputs["has_laplacian_s"]).astype(bool)
    has_t = np.asarray(inputs["has_laplacian_t"]).astype(bool)
    d_s = np.empty(B, dtype=np.float64)
    d_t = np.empty(B, dtype=np.float64)
    for b in range(B):
        o = np.asarray(core_outs[b], dtype=np.float64) / (F_SCALE * F_SCALE)
        d_s[b] = o[:, 0:4].sum() / D
        d_t[b] = o[:, 4:8].sum() / D
    per_graph = 0.5 * (d_s + d_t)
    valid = np.logical_and(has_s, has_t)
    count = valid.sum()
    total = per_graph[valid].sum()
    value = total / max(count, 1.0) if count > 0 else 0.0
    return np.array(value, dtype=np.float32)


def _run(inputs, trace=False, tmpdir=None):
    nc = _get_nc()
    in_maps = _shard_inputs(inputs)
    res = run_bass_kernel_spmd(nc, in_maps, list(range(B)), trace=trace, tmpdir=tmpdir)
    out = _finish([res.results[b]["out"] for b in range(B)], inputs)
    return out, res


def kernel(**inputs):
    out, _ = _run(inputs, trace=False)
    return out


# revision 24
# speedup vs baseline: 1.0009x; 1.0009x over previous
"""DirichletLoss kernel for 8 trn2 NeuronCores.

Math: per graph b, per channel d:
    de[d] = f_d^T L f_d  with f = row-normalized h.

v5 (fp8 one-pass, host-prepacked operands): L is quantized RAW to fp8
e4m3 on the host (4x less HBM traffic; raw N(0,1) entries sit in
e4m3's sweet spot -- folding the norms into L instead lands in the
denormal range and costs 3x accuracy). Normalization is folded into the
stationary side on the host: f16 = fp16(8 * h/||h||row), split into two
fp8 planes (q8a = e4m3(f16), q8b = e4m3(f16 - q8a)) to cancel the
stationary quantization error; the x8 pre-scale keeps both planes out
of e4m3 denormals. Host ships the planes pre-packed [p, k, two, d] and
f^T pre-transposed [64, 2048] f16, so the device does NO casts and NO
PE transposes -- its PE stream is exactly 128 DoubleRow matmuls: both
planes contract against each L row in a single pass via a stride-0
moving AP (out[d,c] = sum_p (q8a+q8b)[p,d] L[p,c]). That is the pure
column floor, 1 column/cycle @2.4GHz = 13.9us/side (measured: weight
switches and moving-tile switches are free; DoubleRow adds rows per
instruction, not column rate; last-slab cadence measured 216ns).

L also ships pre-shuffled into the SBUF image layout [p, k, c], so
every slab DMA is one contiguous 8KB-per-partition descriptor run
(sustains ~400 B/ns vs ~295 with 2KB gather descriptors). All slab
DMAs issue on the sync(SP) queue (splitting across queues measures
worse); small operands issue on the idle ACT queue. Slab sizes ramp up
so the gap-free PE stream starts as early as possible.

Epilogue: fused multiply+reduce on DVE per PSUM bank (only DVE reads
PSUM; gpsimd cannot access PSUM), accumulated straight into the output
tile; host divides by 8^2 and finishes the masked mean. End-to-end
error is the deterministic L-quant term (~6e-3 rel on the fixed seed
vs the 2e-2 gate, host-emulated exactly). Sharding: graph b -> core b.
"""

import numpy as np
import ml_dtypes

import concourse.bacc as bacc
import concourse.bass as bass
import concourse.mybir as mybir
import concourse.tile as tile
from concourse.bass_utils import run_bass_kernel_spmd

B = 8
N = 2048
D = 64
P = 128
NCHUNK = N // P   # 16 contraction chunks of 128 rows
MM_N = 512        # PSUM bank: 512 f32 out columns per matmul
NI = N // MM_N    # 4 output column blocks
F32 = mybir.dt.float32
F16 = mybir.dt.float16
FP8 = mybir.dt.float8e4
E4NP = ml_dtypes.float8_e4m3

# --- tunables -------------------------------------------------------------
SLAB_BUFS = 10  # one buffer per slab: no recycling waits on the sync queue
# Ramp-up slab sizes: small first slabs so the first matmuls start as soon
# as possible (the PE stream is gap-free and ends at start + 30.6us, so
# every us of earlier start is a us off the total).
SLABS = [128, 256, 512, 512, 640]
F_SCALE = 8.0     # keeps f16 and both fp8 planes out of e4m3 denormals
# --------------------------------------------------------------------------


def _emit_body(nc, tc, pools, aps):
    (qpool, ftpool, slabpool, psumpool, smallpool, outpool) = pools
    Ls, q8s, fTs, Lt, q8t, fTt, out = aps

    # accum targets: column side*4 + bank; host sums the 4 pieces per side
    out_sb = outpool.tile([D, 8], F32, tag="out_sb")

    side_small = ((q8s, fTs), (q8t, fTt))

    for side, L_ap in enumerate((Ls, Lt)):
        # small operands issue on the (idle) ACT queue, concurrent with the
        # sync queue's slab issues
        q_ap, fT_ap = side_small[side]
        f8 = qpool.tile([P, NCHUNK, 2, D], FP8, tag="f8")
        nc.scalar.dma_start(
            out=f8[:].rearrange("p k two d -> p (k two d)"), in_=q_ap[:, :]
        )
        fT_sb = ftpool.tile([D, N], F16, tag="fT_sb")
        nc.scalar.dma_start(out=fT_sb[:], in_=fT_ap[:, :])
        P_ps = psumpool.tile([D, N], F32, tag="ps", name="P_ps")
        row0 = 0
        for si, rows in enumerate(SLABS):
            n_blk = rows // P
            slab = slabpool.tile([P, n_blk * N], FP8, tag="slab")
            # L is host-packed [p, k, c] (k = row-block), so a slab is one
            # contiguous 8KB-per-partition descriptor run. All slabs issue
            # on the sync(SP) queue: per-side or alternating splits across
            # queues measured consistently worse.
            k0 = row0 // P
            nc.sync.dma_start(
                out=slab[:],
                in_=L_ap[:, k0 * N : (k0 + n_blk) * N],
            )
            for n in range(n_blk):
                k = row0 // P + n  # global chunk index
                for i in range(NI):
                    # stride-0 "two" dim: both planes contract the same rows
                    mv = slab[:, n * N + i * MM_N : n * N + (i + 1) * MM_N]
                    mv_b = bass.AP(
                        mv.tensor, mv.offset,
                        [list(mv.ap[0]), [0, 2], [1, MM_N]],
                    )
                    nc.tensor.matmul(
                        P_ps[:, i * MM_N : (i + 1) * MM_N],
                        f8[:, k, :, :],
                        mv_b,
                        start=(k == 0),
                        stop=(k == NCHUNK - 1),
                        perf_mode=mybir.MatmulPerfMode.DoubleRow,
                    )
            row0 += rows

        # de pieces: one fused DVE op per PSUM bank via
        # scalar_tensor_tensor accum_out, straight into out_sb (gpsimd
        # cannot access PSUM; only DVE reads it for tensor ops). NOTE: the
        # dedicated tensor_tensor_reduce op crashes the exec unit on HW;
        # the TensorScalarPtr accum path is the safe fused form.
        for i in range(NI):
            tmp = smallpool.tile([D, MM_N], F32, tag=f"ttr_tmp{i % 2}",
                                 name=f"tmp{i}")
            nc.vector.scalar_tensor_tensor(
                out=tmp[:],
                in0=P_ps[:, i * MM_N : (i + 1) * MM_N],
                scalar=1.0,
                in1=fT_sb[:, i * MM_N : (i + 1) * MM_N],
                op0=mybir.AluOpType.mult,
                op1=mybir.AluOpType.mult,
                accum_out=out_sb[:, 4 * side + i : 4 * side + i + 1],
            )

    nc.sync.dma_start(out=out[:], in_=out_sb[:])


def build_program():
    nc = bacc.Bacc(trn_type="TRN2")

    Ls = nc.declare_dram_parameter("Ls", [P, NCHUNK * N], FP8, isOutput=False)
    q8s = nc.declare_dram_parameter("q8s", [P, NCHUNK * 2 * D], FP8, isOutput=False)
    fTs = nc.declare_dram_parameter("fTs", [D, N], F16, isOutput=False)
    Lt = nc.declare_dram_parameter("Lt", [P, NCHUNK * N], FP8, isOutput=False)
    q8t = nc.declare_dram_parameter("q8t", [P, NCHUNK * 2 * D], FP8, isOutput=False)
    fTt = nc.declare_dram_parameter("fTt", [D, N], F16, isOutput=False)
    out = nc.declare_dram_parameter("out", [D, 8], F32, isOutput=True)
    aps = (Ls, q8s, fTs, Lt, q8t, fTt, out)

    with tile.TileContext(nc) as tc:
        with (
            tc.tile_pool(name="qp", bufs=2) as qpool,
            tc.tile_pool(name="ftp", bufs=2) as ftpool,
            tc.tile_pool(name="slab", bufs=SLAB_BUFS) as slabpool,
            tc.tile_pool(name="psum", bufs=2, space="PSUM") as psumpool,
            tc.tile_pool(name="small", bufs=2) as smallpool,
            tc.tile_pool(name="outp", bufs=1) as outpool,
        ):
            pools = (qpool, ftpool, slabpool, psumpool, smallpool, outpool)
            _emit_body(nc, tc, pools, aps)

    nc.compile()
    return nc


_CACHED_NC = None


def _get_nc():
    global _CACHED_NC
    if _CACHED_NC is None:
        _CACHED_NC = build_program()
    return _CACHED_NC


def _prep_side(lap, h):
    """Per-graph host prep: Lq e4m3 packed [p, k, c], fp8 plane pair, fT f16."""
    Lq = np.asarray(lap, dtype=np.float32).reshape(B, N, N).astype(E4NP)
    # pack to the SBUF image: Lpk[b, p, k, c] = L[b, k*128+p, c]
    Lq = np.ascontiguousarray(
        Lq.reshape(B, NCHUNK, P, N).transpose(0, 2, 1, 3)
    ).reshape(B, P, NCHUNK * N)
    hh = np.asarray(h, dtype=np.float64).reshape(B, N, D)
    n = np.sqrt((hh * hh).sum(axis=2, keepdims=True))
    f16 = (hh / np.maximum(n, 1e-12) * F_SCALE).astype(np.float16)
    f32 = f16.astype(np.float32)
    q8a = f32.astype(E4NP)
    q8b = (f32 - q8a.astype(np.float32)).astype(E4NP)
    # pack [B, p, k, two, d]: row j = k*128 + p
    qab = np.stack([q8a, q8b], axis=2)            # [B, N, 2, D]
    qab = qab.reshape(B, NCHUNK, P, 2, D).transpose(0, 2, 1, 3, 4)
    q8 = np.ascontiguousarray(qab).reshape(B, P, NCHUNK * 2 * D)
    fT = np.ascontiguousarray(f16.transpose(0, 2, 1))  # [B, D, N]
    return Lq, q8, fT


def _shard_inputs(inputs):
    Lqs, q8s, fTs = _prep_side(inputs["laplacian_s"], inputs["h_s"])
    Lqt, q8t, fTt = _prep_side(inputs["laplacian_t"], inputs["h_t"])
    return [
        {
            "Ls": Lqs[b], "q8s": q8s[b], "fTs": fTs[b],
            "Lt": Lqt[b], "q8t": q8t[b], "fTt": fTt[b],
        }
        for b in range(B)
    ]


def _finish(core_outs, inputs):
    has_s = np.asarray(inputs["has_laplacian_s"]).astype(bool)
    has_t = np.asarray(inputs["has_laplacian_t"]).astype(bool)
    d_s = np.empty(B, dtype=np.float64)
    d_t = np.empty(B, dtype=np.float64)
    for b in range(B):
        o = np.asarray(core_outs[b], dtype=np.float64) / (F_SCALE * F_SCALE)
        d_s[b] = o[:, 0:4].sum() / D
        d_t[b] = o[:, 4:8].sum() / D
    per_graph = 0.5 * (d_s + d_t)
    valid = np.logical_and(has_s, has_t)
    count = valid.sum()
    total = per_graph[valid].sum()
    value = total / max(count, 1.0) if count > 0 else 0.0
    return np.array(value, dtype=np.float32)


def _run(inputs, trace=False, tmpdir=None):
    nc = _get_nc()
    in_maps = _shard_inputs(inputs)
    res = run_bass_kernel_spmd(nc, in_maps, list(range(B)), trace=trace, tmpdir=tmpdir)
    out = _finish([res.results[b]["out"] for b in range(B)], inputs)
    return out, res


def kernel(**inputs):
    out, _ = _run(inputs, trace=False)
    return out


# revision 25
# speedup vs baseline: 1.0273x; 1.0264x over previous
"""DirichletLoss kernel for 8 trn2 NeuronCores.

Math: per graph b, per channel d:
    de[d] = f_d^T L f_d  with f = row-normalized h.

v5 (fp8 one-pass, host-prepacked operands): L is quantized RAW to fp8
e4m3 on the host (4x less HBM traffic; raw N(0,1) entries sit in
e4m3's sweet spot -- folding the norms into L instead lands in the
denormal range and costs 3x accuracy). Normalization is folded into the
stationary side on the host: f16 = fp16(8 * h/||h||row), split into two
fp8 planes (q8a = e4m3(f16), q8b = e4m3(f16 - q8a)) to cancel the
stationary quantization error; the x8 pre-scale keeps both planes out
of e4m3 denormals. Host ships the planes pre-packed [p, k, two, d] and
f^T pre-transposed [64, 2048] f16, so the device does NO casts and NO
PE transposes -- its PE stream is exactly 128 DoubleRow matmuls: both
planes contract against each L row in a single pass via a stride-0
moving AP (out[d,c] = sum_p (q8a+q8b)[p,d] L[p,c]). That is the pure
column floor, 1 column/cycle @2.4GHz = 13.9us/side (measured: weight
switches and moving-tile switches are free; DoubleRow adds rows per
instruction, not column rate; last-slab cadence measured 216ns).

L also ships pre-shuffled into the SBUF image layout [p, k, c], so
every slab DMA is one contiguous 8KB-per-partition descriptor run
(sustains ~400 B/ns vs ~295 with 2KB gather descriptors). All slab
DMAs issue on the sync(SP) queue (splitting across queues measures
worse); small operands issue on the idle ACT queue. Slab sizes ramp up
so the gap-free PE stream starts as early as possible.

Epilogue: fused multiply+reduce on DVE per PSUM bank (only DVE reads
PSUM; gpsimd cannot access PSUM), accumulated straight into the output
tile; host divides by 8^2 and finishes the masked mean. End-to-end
error is the deterministic L-quant term (~6e-3 rel on the fixed seed
vs the 2e-2 gate, host-emulated exactly). Sharding: graph b -> core b.
"""

import numpy as np
import ml_dtypes

import concourse.bacc as bacc
import concourse.bass as bass
import concourse.mybir as mybir
import concourse.tile as tile
from concourse.bass_utils import run_bass_kernel_spmd

B = 8
N = 2048
D = 64
P = 128
NCHUNK = N // P   # 16 contraction chunks of 128 rows
MM_N = 512        # PSUM bank: 512 f32 out columns per matmul
NI = N // MM_N    # 4 output column blocks
F32 = mybir.dt.float32
F16 = mybir.dt.float16
FP8 = mybir.dt.float8e4
E4NP = ml_dtypes.float8_e4m3

# --- tunables -------------------------------------------------------------
SLAB_BUFS = 10  # one buffer per slab: no recycling waits on the sync queue
# Ramp-up slab sizes: small first slabs so the first matmuls start as soon
# as possible (the PE stream is gap-free and ends at start + 30.6us, so
# every us of earlier start is a us off the total).
SLABS = [128, 256, 512, 512, 384, 256]
F_SCALE = 8.0     # keeps f16 and both fp8 planes out of e4m3 denormals
# --------------------------------------------------------------------------


def _emit_body(nc, tc, pools, aps):
    (qpool, ftpool, slabpool, psumpool, smallpool, outpool) = pools
    Ls, q8s, fTs, Lt, q8t, fTt, out = aps

    # accum targets: column side*4 + bank; host sums the 4 pieces per side
    out_sb = outpool.tile([D, 8], F32, tag="out_sb")

    side_small = ((q8s, fTs), (q8t, fTt))

    for side, L_ap in enumerate((Ls, Lt)):
        # small operands issue on the (idle) ACT queue, concurrent with the
        # sync queue's slab issues
        q_ap, fT_ap = side_small[side]
        f8 = qpool.tile([P, NCHUNK, 2, D], FP8, tag="f8")
        nc.scalar.dma_start(
            out=f8[:].rearrange("p k two d -> p (k two d)"), in_=q_ap[:, :]
        )
        fT_sb = ftpool.tile([D, N], F16, tag="fT_sb")
        nc.scalar.dma_start(out=fT_sb[:], in_=fT_ap[:, :])
        P_ps = psumpool.tile([D, N], F32, tag="ps", name="P_ps")
        row0 = 0
        for si, rows in enumerate(SLABS):
            n_blk = rows // P
            slab = slabpool.tile([P, n_blk * N], FP8, tag="slab")
            # L is host-packed [p, k, c] (k = row-block), so a slab is one
            # contiguous 8KB-per-partition descriptor run. All slabs issue
            # on the sync(SP) queue: per-side or alternating splits across
            # queues measured consistently worse.
            k0 = row0 // P
            nc.sync.dma_start(
                out=slab[:],
                in_=L_ap[:, k0 * N : (k0 + n_blk) * N],
            )
            for n in range(n_blk):
                k = row0 // P + n  # global chunk index
                for i in range(NI):
                    # stride-0 "two" dim: both planes contract the same rows
                    mv = slab[:, n * N + i * MM_N : n * N + (i + 1) * MM_N]
                    mv_b = bass.AP(
                        mv.tensor, mv.offset,
                        [list(mv.ap[0]), [0, 2], [1, MM_N]],
                    )
                    nc.tensor.matmul(
                        P_ps[:, i * MM_N : (i + 1) * MM_N],
                        f8[:, k, :, :],
                        mv_b,
                        start=(k == 0),
                        stop=(k == NCHUNK - 1),
                        perf_mode=mybir.MatmulPerfMode.DoubleRow,
                    )
            row0 += rows

        # de: ONE fused DVE op over the whole [64, 2048] PSUM region
        # (APs span banks) via scalar_tensor_tensor accum_out, straight
        # into out_sb -- pays the PSUM-access/seq overhead once instead of
        # 4x (gpsimd cannot access PSUM; only DVE reads it for tensor
        # ops). NOTE: the dedicated tensor_tensor_reduce op crashes the
        # exec unit on HW; the TensorScalarPtr accum path is the safe
        # fused form.
        tmp = smallpool.tile([D, N], F32, tag="ttr_tmp", name="tmp")
        nc.vector.scalar_tensor_tensor(
            out=tmp[:],
            in0=P_ps[:],
            scalar=1.0,
            in1=fT_sb[:],
            op0=mybir.AluOpType.mult,
            op1=mybir.AluOpType.mult,
            accum_out=out_sb[:, 4 * side : 4 * side + 1],
        )

    nc.sync.dma_start(out=out[:], in_=out_sb[:])


def build_program():
    nc = bacc.Bacc(trn_type="TRN2")

    Ls = nc.declare_dram_parameter("Ls", [P, NCHUNK * N], FP8, isOutput=False)
    q8s = nc.declare_dram_parameter("q8s", [P, NCHUNK * 2 * D], FP8, isOutput=False)
    fTs = nc.declare_dram_parameter("fTs", [D, N], F16, isOutput=False)
    Lt = nc.declare_dram_parameter("Lt", [P, NCHUNK * N], FP8, isOutput=False)
    q8t = nc.declare_dram_parameter("q8t", [P, NCHUNK * 2 * D], FP8, isOutput=False)
    fTt = nc.declare_dram_parameter("fTt", [D, N], F16, isOutput=False)
    out = nc.declare_dram_parameter("out", [D, 8], F32, isOutput=True)
    aps = (Ls, q8s, fTs, Lt, q8t, fTt, out)

    with tile.TileContext(nc) as tc:
        with (
            tc.tile_pool(name="qp", bufs=2) as qpool,
            tc.tile_pool(name="ftp", bufs=2) as ftpool,
            tc.tile_pool(name="slab", bufs=SLAB_BUFS) as slabpool,
            tc.tile_pool(name="psum", bufs=2, space="PSUM") as psumpool,
            tc.tile_pool(name="small", bufs=2) as smallpool,
            tc.tile_pool(name="outp", bufs=1) as outpool,
        ):
            pools = (qpool, ftpool, slabpool, psumpool, smallpool, outpool)
            _emit_body(nc, tc, pools, aps)

    nc.compile()
    return nc


_CACHED_NC = None


def _get_nc():
    global _CACHED_NC
    if _CACHED_NC is None:
        _CACHED_NC = build_program()
    return _CACHED_NC


def _prep_side(lap, h):
    """Per-graph host prep: Lq e4m3 packed [p, k, c], fp8 plane pair, fT f16."""
    Lq = np.asarray(lap, dtype=np.float32).reshape(B, N, N).astype(E4NP)
    # pack to the SBUF image: Lpk[b, p, k, c] = L[b, k*128+p, c]
    Lq = np.ascontiguousarray(
        Lq.reshape(B, NCHUNK, P, N).transpose(0, 2, 1, 3)
    ).reshape(B, P, NCHUNK * N)
    hh = np.asarray(h, dtype=np.float64).reshape(B, N, D)
    n = np.sqrt((hh * hh).sum(axis=2, keepdims=True))
    f16 = (hh / np.maximum(n, 1e-12) * F_SCALE).astype(np.float16)
    f32 = f16.astype(np.float32)
    q8a = f32.astype(E4NP)
    q8b = (f32 - q8a.astype(np.float32)).astype(E4NP)
    # pack [B, p, k, two, d]: row j = k*128 + p
    qab = np.stack([q8a, q8b], axis=2)            # [B, N, 2, D]
    qab = qab.reshape(B, NCHUNK, P, 2, D).transpose(0, 2, 1, 3, 4)
    q8 = np.ascontiguousarray(qab).reshape(B, P, NCHUNK * 2 * D)
    fT = np.ascontiguousarray(f16.transpose(0, 2, 1))  # [B, D, N]
    return Lq, q8, fT


def _shard_inputs(inputs):
    Lqs, q8s, fTs = _prep_side(inputs["laplacian_s"], inputs["h_s"])
    Lqt, q8t, fTt = _prep_side(inputs["laplacian_t"], inputs["h_t"])
    return [
        {
            "Ls": Lqs[b], "q8s": q8s[b], "fTs": fTs[b],
            "Lt": Lqt[b], "q8t": q8t[b], "fTt": fTt[b],
        }
        for b in range(B)
    ]


def _finish(core_outs, inputs):
    has_s = np.asarray(inputs["has_laplacian_s"]).astype(bool)
    has_t = np.asarray(inputs["has_laplacian_t"]).astype(bool)
    d_s = np.empty(B, dtype=np.float64)
    d_t = np.empty(B, dtype=np.float64)
    for b in range(B):
        o = np.asarray(core_outs[b], dtype=np.float64) / (F_SCALE * F_SCALE)
        d_s[b] = o[:, 0:4].sum() / D
        d_t[b] = o[:, 4:8].sum() / D
    per_graph = 0.5 * (d_s + d_t)
    valid = np.logical_and(has_s, has_t)
    count = valid.sum()
    total = per_graph[valid].sum()
    value = total / max(count, 1.0) if count > 0 else 0.0
    return np.array(value, dtype=np.float32)


def _run(inputs, trace=False, tmpdir=None):
    nc = _get_nc()
    in_maps = _shard_inputs(inputs)
    res = run_bass_kernel_spmd(nc, in_maps, list(range(B)), trace=trace, tmpdir=tmpdir)
    out = _finish([res.results[b]["out"] for b in range(B)], inputs)
    return out, res


def kernel(**inputs):
    out, _ = _run(inputs, trace=False)
    return out
